# revision 1
# baseline (speedup 1.0000x reference)
"""AGNNConv (single-head attention message passing) on 8 TRN2 NeuronCores.

Reference computation (N=100000 nodes, fixed degree 16, D=64):
    X_prime = X @ W                                  # [N, 64]
    e[n,k]  = <X_prime[n], X_prime[ci[n,k]]> * s     # s = attention_w[0,0]
    out[n]  = sum_k e[n,k] * X_prime[ci[n,k]]        # [N, 64]

Sharding: nodes split 12500/core across 8 cores. Each core computes its
X_prime shard (and a pre-scaled copy via W*s), AllGathers the bf16 table,
then gathers its nodes' 16 neighbor rows per 128-node tile with an
indirect DMA and does the dot/weight/aggregate on the vector engine.
"""

import sys

import ml_dtypes
import numpy as np

if "/opt/trn_rl_repo" not in sys.path:
    sys.path.insert(0, "/opt/trn_rl_repo")

N_NODES = 100000
DEG = 16
D = 64
CORES = 8
NPC = N_NODES // CORES  # 12500
P = 128
NTILES = (NPC + P - 1) // P  # 98
NPAD = NTILES * P  # 12544


def build_nc(n_nodes=N_NODES, npc=NPC, deg=DEG, d=D, cores=CORES, lowering=False):
    from concourse import bacc, bass, mybir, tile

    ntiles = (npc + P - 1) // P
    npad = ntiles * P

    f32 = mybir.dt.float32
    bf16 = mybir.dt.bfloat16
    i32 = mybir.dt.int32

    nc = bacc.Bacc(
        "TRN2", target_bir_lowering=lowering, debug=False, num_devices=cores
    )

    # xT carries [X_shard.T | W | W*s] so the matmuls depend on ONE input DMA
    # (the Matmult LdWeights slot only fits a single semaphore wait).
    xT = nc.declare_dram_parameter("xT", [d, npad + 2 * d], f32, isOutput=False)
    # idx16: per-tile dma_gather index image (remapped node // 4, int16,
    # wrapped-16 layout). masks: per-(node, slot) one-hot of node % 4, bf16.
    i16 = mybir.dt.int16
    idx16 = nc.declare_dram_parameter("idx16", [P, ntiles * P], i16, isOutput=False)
    masks = nc.declare_dram_parameter(
        "masks", [P, ntiles * 4 * deg], bf16, isOutput=False
    )
    out_ext = nc.declare_dram_parameter("out", [npad, d], f32, isOutput=True)

    # Padded to npad rows so ONE DMA fills each cc_in half (the collective-
    # trigger ISA struct only fits a single semaphore wait). Neighbor indices
    # are host-remapped to this padded, half-split row numbering.
    half_tiles = max(1, ntiles // 4)
    half = half_tiles * P
    cc_in_a = nc.dram_tensor("cc_in_a", [half, d], bf16)
    cc_in_b = nc.dram_tensor("cc_in_b", [npad - half, d], bf16)
    cc_out = nc.dram_tensor(
        "cc_out", [cores * npad // 4, 4 * d], bf16, addr_space="Shared"
    )

    with tile.TileContext(nc) as tc:
        with (
            tc.tile_pool(name="const", bufs=1) as cpool,
            tc.tile_pool(name="psum", bufs=4, space="PSUM") as psum,
            tc.tile_pool(name="g", bufs=4) as gpool,
            tc.tile_pool(name="g4", bufs=3) as g4pool,
            tc.tile_pool(name="prod", bufs=2) as ppool,
            tc.tile_pool(name="q", bufs=2) as qpool,
            tc.tile_pool(name="e", bufs=3) as epool,
            tc.tile_pool(name="o", bufs=3) as opool,
        ):
            xT_sb = cpool.tile([d, npad + 2 * d], f32, tag="xT_sb")
            xp_bf = cpool.tile([P, ntiles * d], bf16, tag="xp_bf")
            sxp_bf = cpool.tile([P, ntiles * d], bf16, tag="sxp_bf")
            idx_sb = cpool.tile([P, ntiles * P], i16, tag="idx_sb")
            msk_sb = cpool.tile([P, ntiles * 4 * deg], bf16, tag="msk_sb")

            nc.sync.dma_start(out=xT_sb[:, :], in_=xT[:, :])
            nc.sync.dma_start(out=idx_sb[:, :], in_=idx16[:, :])
            nc.sync.dma_start(out=msk_sb[:, :], in_=masks[:, :])

            # X_prime shard (bf16) and pre-scaled X_prime*s shard, one matmul
            # per tile against the concatenated [W | W*s] (adjacent in SBUF).
            ww2 = xT_sb[:, npad : npad + 2 * d]
            for t in range(ntiles):
                ps1 = psum.tile([P, 2 * d], f32, tag="ps1")
                nc.tensor.matmul(
                    ps1[:, :],
                    xT_sb[:, t * P : (t + 1) * P],
                    ww2,
                    start=True,
                    stop=True,
                )
                nc.vector.tensor_copy(out=xp_bf[:, t * d : (t + 1) * d], in_=ps1[:, 0:d])
                nc.vector.tensor_copy(
                    out=sxp_bf[:, t * d : (t + 1) * d], in_=ps1[:, d : 2 * d]
                )

            # Shard -> internal DRAM -> AllGather (two halves; the first AG
            # overlaps the second half of the matmul phase).
            nc.sync.dma_start(
                out=cc_in_a[:, :].rearrange("(t p) f -> p t f", p=P),
                in_=xp_bf[:, 0 : half_tiles * d].rearrange(
                    "p (t f) -> p t f", t=half_tiles
                ),
            )
            nc.gpsimd.collective_compute(
                "AllGather",
                mybir.AluOpType.bypass,
                replica_groups=[list(range(cores))],
                ins=[cc_in_a.ap()],
                outs=[cc_out[0 : cores * half // 4, :]],
            )
            nc.sync.dma_start(
                out=cc_in_b[:, :].rearrange("(t p) f -> p t f", p=P),
                in_=xp_bf[:, half_tiles * d :].rearrange(
                    "p (t f) -> p t f", t=ntiles - half_tiles
                ),
            )
            nc.gpsimd.collective_compute(
                "AllGather",
                mybir.AluOpType.bypass,
                replica_groups=[list(range(cores))],
                ins=[cc_in_b.ap()],
                outs=[cc_out[cores * half // 4 :, :]],
            )

            # The SWDGE queue descriptor fits only ONE semaphore wait, but the
            # first gather depends on both the collective (cc_out) and the idx
            # DMA. Absorb each wait into the SWDGE proc with a tiny DMA first.
            scr = cpool.tile([1, d], bf16, tag="scr")
            scr2 = cpool.tile([1, deg], i16, tag="scr2")
            d1 = nc.gpsimd.dma_start(out=scr[:, :], in_=cc_out[0:1, 0:d])
            d2 = nc.gpsimd.dma_start(out=scr2[:, :], in_=idx_sb[0:1, 0:deg])

            # Gather + edge compute, one 128-node tile at a time.
            from concourse.tile import add_dep_helper

            for t in range(ntiles):
                rows = min(P, npc - t * P)
                G4 = g4pool.tile([P, deg * 4 * d], bf16, tag="G4")
                nc.gpsimd.dma_gather(
                    G4[:, :].rearrange("p (k f) -> p k f", k=deg),
                    cc_out[:, :],
                    idx_sb[:, t * P : (t + 1) * P],
                    deg * P,
                    deg * P,
                    4 * d,
                    single_packet=False,
                )
                # select the right node-quarter of each gathered 4-node row
                Gq = []
                for q in range(4):
                    Pq = ppool.tile([P, deg * d], bf16, tag=f"Pq{q % 2}")
                    nc.vector.tensor_tensor(
                        out=Pq[0:rows, :].rearrange("p (k f) -> p k f", k=deg),
                        in0=G4[0:rows, :]
                        .rearrange("p (k f) -> p k f", k=deg)[:, :, q * d : (q + 1) * d],
                        in1=msk_sb[0:rows, t * 4 * deg : (t + 1) * 4 * deg]
                        .rearrange("p (k q) -> p k q", q=4)[:, :, q : q + 1]
                        .broadcast_to([rows, deg, d]),
                        op=mybir.AluOpType.mult,
                    )
                    Gq.append(Pq)
                    if q == 1:
                        A01 = qpool.tile([P, deg * d], bf16, tag="A01")
                        nc.vector.tensor_tensor(
                            out=A01[0:rows, :], in0=Gq[0][0:rows, :],
                            in1=Gq[1][0:rows, :], op=mybir.AluOpType.add,
                        )
                    if q == 3:
                        A23 = qpool.tile([P, deg * d], bf16, tag="A23")
                        nc.vector.tensor_tensor(
                            out=A23[0:rows, :], in0=Gq[2][0:rows, :],
                            in1=Gq[3][0:rows, :], op=mybir.AluOpType.add,
                        )
                G = gpool.tile([P, deg * d], bf16, tag="G")
                nc.vector.tensor_tensor(
                    out=G[0:rows, :], in0=A01[0:rows, :], in1=A23[0:rows, :],
                    op=mybir.AluOpType.add,
                )
                Gv = G[0:rows, :].rearrange("p (k f) -> p k f", k=deg)
                Pt = ppool.tile([P, deg * d], bf16, tag="Pt")
                nc.vector.tensor_tensor(
                    out=Pt[0:rows, :].rearrange("p (k f) -> p k f", k=deg),
                    in0=Gv,
                    in1=sxp_bf[0:rows, t * d : (t + 1) * d]
                    .unsqueeze(1)
                    .broadcast_to([rows, deg, d]),
                    op=mybir.AluOpType.mult,
                )
                e = epool.tile([P, deg], bf16, tag="e")
                with nc.allow_low_precision(reason="bf16 edge attn within tolerance"):
                    nc.vector.tensor_reduce(
                        out=e[0:rows, :],
                        in_=Pt[0:rows, :].rearrange("p (k f) -> p k f", k=deg),
                        axis=mybir.AxisListType.X,
                        op=mybir.AluOpType.add,
                    )
                Qt = qpool.tile([P, deg * d], bf16, tag="Qt")
                nc.vector.tensor_tensor(
                    out=Qt[0:rows, :].rearrange("p (k f) -> p k f", k=deg),
                    in0=Gv,
                    in1=e[0:rows, :].unsqueeze(2).broadcast_to([rows, deg, d]),
                    op=mybir.AluOpType.mult,
                )
                o = opool.tile([P, d], f32, tag="o")
                nc.vector.tensor_reduce(
                    out=o[0:rows, :],
                    in_=Qt[0:rows, :].rearrange("p (k f) -> p f k", k=deg),
                    axis=mybir.AxisListType.X,
                    op=mybir.AluOpType.add,
                )
                nc.sync.dma_start(
                    out=out_ext[t * P : t * P + rows, :], in_=o[0:rows, :]
                )

    nc.compile()
    return nc


def make_in_maps(X, weights, attention_w, column_index, n_nodes=N_NODES, cores=CORES):
    npc = n_nodes // cores
    ntiles = (npc + P - 1) // P
    npad = ntiles * P
    s = float(np.asarray(attention_w).reshape(-1)[0])
    w = np.asarray(weights, dtype=np.float32)
    ci_all = np.asarray(column_index, dtype=np.int32).reshape(n_nodes, DEG)
    in_maps = []
    for c in range(cores):
        r0, r1 = c * npc, (c + 1) * npc
        xT = np.zeros((D, npad + 2 * D), dtype=np.float32)
        xT[:, :npc] = np.asarray(X[r0:r1], dtype=np.float32).T
        xT[:, npad : npad + D] = w
        xT[:, npad + D : npad + 2 * D] = w * s
        rem = np.zeros((npad, DEG), dtype=np.int64)
        ci_shard = ci_all[r0:r1].astype(np.int64)
        # remap node id -> row in the split, npad-padded AllGather table
        half = max(1, npad // P // 4) * P
        own = ci_shard // npc
        loc = ci_shard % npc
        rem[:npc] = np.where(
            loc < half,
            own * half + loc,
            cores * half + own * (npad - half) + (loc - half),
        )
        ntiles = npad // P
        ci4 = (rem // 4).astype(np.int16)
        qq = (rem % 4).astype(np.int64)
        # dma_gather index image: image[16g+pp, t*128+s] = flat_t[s*16+pp],
        # flat_t[j2*128+p] = ci4[t*128+p, j2]
        flat = ci4.reshape(ntiles, P, DEG).transpose(0, 2, 1).reshape(ntiles, P * DEG)
        img16 = flat.reshape(ntiles, P, 16).transpose(2, 0, 1).reshape(16, ntiles * P)
        idx_img = np.tile(img16, (8, 1))
        # mask image: [p, t*64 + k*4 + q] one-hot of node%4
        mk = np.zeros((npad, DEG, 4), dtype=ml_dtypes.bfloat16)
        np.put_along_axis(mk, qq[:, :, None], 1.0, axis=2)
        msk_img = (
            mk.reshape(ntiles, P, 4 * DEG)
            .transpose(1, 0, 2)
            .reshape(P, ntiles * 4 * DEG)
        )
        in_maps.append(
            {
                "xT": np.ascontiguousarray(xT),
                "idx16": np.ascontiguousarray(idx_img),
                "masks": np.ascontiguousarray(msk_img),
            }
        )
    return in_maps


_NC_CACHE = {}


def _get_nc():
    key = (N_NODES, NPC)
    if key not in _NC_CACHE:
        _NC_CACHE[key] = build_nc()
    return _NC_CACHE[key]


def run(X, weights, attention_w, column_index, trace=False, **trace_kwargs):
    from concourse import bass_utils

    nc = _get_nc()
    in_maps = make_in_maps(X, weights, attention_w, column_index)
    res = bass_utils.run_bass_kernel_spmd(
        nc, in_maps, core_ids=list(range(CORES)), trace=trace, **trace_kwargs
    )
    outs = [np.asarray(res.results[c]["out"][:NPC]) for c in range(CORES)]
    return np.concatenate(outs, axis=0).astype(np.float32), res


def kernel(
    X,
    weights,
    attention_w,
    row_pointers,
    column_index,
    blockPartition,
    edgeToColumn,
    edgeToRow,
    **_unused,
):
    out, _ = run(X, weights, attention_w, column_index)
    return out

